# revision 3
# baseline (speedup 1.0000x reference)
"""Int4-weight (groupwise-dequant) linear with dynamic per-token int8 activation
fake-quant, for 8 trn2 NeuronCores.

Math (per reference):
    w_dq[o,i]  = (w[o,i] - zeros[o, i//32]) * scales[o, i//32]
    amax[t]    = max_i |x[t,i]|;  a_scale[t] = max(amax,1e-12)/127
    q[t,i]     = clip(round(x[t,i]/a_scale[t]), -128, 127)
    out[t,o]   = sum_i (q[t,i]*a_scale[t]) * w_dq[o,i]

Sharding: column-parallel over out_features (padded 11008->11264, 1408/core),
activations replicated. Each core computes out[:, c*1408:(c+1)*1408]; host
concatenates and drops the padding.

Device strategy per core:
  - Phase A: dequantize the int8 weight shard to bf16 on DVE (w and zeros/scales
    enter as exact small integers; only the product rounds once to bf16), then
    DMA-xbar-transpose 128x128 blocks into a SBUF-resident [IN, 1408] bf16
    operand laid out K-major for the PE.
  - Phase B (per 128-token tile): load x fp32, DVE abs-max reduce -> a_scale,
    inv = 1/a_scale; ACT computes x*inv + 1.5*2^23 (fp32 magic rounding), DVE
    subtracts the magic and casts to bf16 (exact integers in [-128,127]); DMA
    transposes to K-major; 32 K-chunk matmuls accumulate fp32 PSUM per <=512
    column tile; ACT applies the per-token a_scale while copying PSUM->SBUF;
    DMA out.
The bf16 activation operand is exact, so the only quantization vs the reference
is the single bf16 rounding of w_dq (~2^-10 relative).
"""

import os
import sys

for _p in ("/opt/trn_rl_repo", "/root/.axon_site/_ro/trn_rl_repo"):
    if os.path.isdir(_p) and _p not in sys.path:
        sys.path.append(_p)

import numpy as np

B, S, IN_DIM, OUT_DIM = 4, 2048, 4096, 11008
GROUP = 32
G = IN_DIM // GROUP          # 128 groups per row
N_CORES = 8
OUT_PAD = 11264              # smallest multiple of 128*8 >= 11008
OUT_C = OUT_PAD // N_CORES   # 1408 out features per core
T_TOK = B * S                # 8192 tokens
MAGIC = 12582912.0           # 1.5 * 2**23: fp32 add/sub rounds to nearest int


def build_bass(t_tok=T_TOK, out_c=OUT_C, in_dim=IN_DIM, repeat=1, xpool_bufs=3,
               psum_bufs=6):
    import concourse.bacc as bacc
    import concourse.mybir as mybir
    import concourse.tile as tile
    from contextlib import ExitStack

    f32, bf16, i8 = mybir.dt.float32, mybir.dt.bfloat16, mybir.dt.int8
    Alu = mybir.AluOpType
    Act = mybir.ActivationFunctionType

    KC = in_dim // 128            # contraction chunks of 128
    OFB = out_c // 128            # out-feature 128-blocks
    ngrp = in_dim // GROUP
    KQ_SIZE = 1024 if in_dim % 1024 == 0 else in_dim
    KQ = in_dim // KQ_SIZE        # weight-dequant pieces along IN
    nsplits = []
    n0 = 0
    while n0 < out_c:
        nw = min(512, out_c - n0)
        nsplits.append((n0, nw))
        n0 += nw

    nc = bacc.Bacc("TRN2", target_bir_lowering=False, debug=False,
                   num_devices=N_CORES)
    x = nc.dram_tensor("x", [t_tok, in_dim], f32, kind="ExternalInput")
    w = nc.dram_tensor("w", [out_c, in_dim], i8, kind="ExternalInput")
    sc = nc.dram_tensor("scales", [out_c, ngrp], f32, kind="ExternalInput")
    zp = nc.dram_tensor("zeros", [out_c, ngrp], f32, kind="ExternalInput")
    out = nc.dram_tensor("out", [t_tok, out_c], f32, kind="ExternalOutput")

    def bcast_inner(ap, n):
        # append a 0-stride inner dim: [128, g] -> [128, g, n] broadcast
        return ap.to_broadcast((*ap.shape, n))

    with tile.TileContext(nc) as tc, ExitStack() as ctx:
        wres = ctx.enter_context(tc.tile_pool(name="wres", bufs=1))
        wstage = ctx.enter_context(tc.tile_pool(name="wstage", bufs=3))
        qparam = ctx.enter_context(tc.tile_pool(name="qparam", bufs=3))
        wtmp = ctx.enter_context(tc.tile_pool(name="wtmp", bufs=3))
        xpool = ctx.enter_context(tc.tile_pool(name="xpool", bufs=xpool_bufs))
        scal = ctx.enter_context(tc.tile_pool(name="scal", bufs=4))
        xqpool = ctx.enter_context(tc.tile_pool(name="xqp", bufs=2))
        xqt = ctx.enter_context(tc.tile_pool(name="xqt", bufs=2))
        opool = ctx.enter_context(tc.tile_pool(name="opool", bufs=2))
        psum = ctx.enter_context(
            tc.tile_pool(name="psum", bufs=psum_bufs, space="PSUM"))

        def body(_it=None):
            wT = wres.tile([128, KC, out_c], bf16, tag="wT")
            # ---- Phase A: dequantize weight shard, transpose to K-major ----
            for kq in range(KQ):
                gs = KQ_SIZE // GROUP
                g0 = kq * gs
                for of in range(OFB):
                    wi = wstage.tile([128, KQ_SIZE], i8, tag="wi")
                    nc.gpsimd.dma_start(
                        wi[:], w[of * 128:(of + 1) * 128,
                                 kq * KQ_SIZE:(kq + 1) * KQ_SIZE])
                    zt = qparam.tile([128, gs], f32, tag="z")
                    st = qparam.tile([128, gs], f32, tag="s")
                    nc.gpsimd.dma_start(
                        zt[:], zp[of * 128:(of + 1) * 128, g0:g0 + gs])
                    nc.gpsimd.dma_start(
                        st[:], sc[of * 128:(of + 1) * 128, g0:g0 + gs])
                    w1 = wtmp.tile([128, KQ_SIZE], bf16, tag="w1")
                    nc.vector.tensor_tensor(
                        w1[:], wi[:], bcast_inner(zt[:], GROUP), Alu.subtract)
                    w2 = wtmp.tile([128, KQ_SIZE], bf16, tag="w2")
                    nc.vector.tensor_tensor(
                        w2[:], w1[:], bcast_inner(st[:], GROUP), Alu.mult)
                    for kk in range(KQ_SIZE // 128):
                        k = kq * (KQ_SIZE // 128) + kk
                        nc.sync.dma_start(
                            wT[:, k, of * 128:(of + 1) * 128],
                            w2[:, kk * 128:(kk + 1) * 128], transpose=True)
            # ---- Phase B: token tiles ----
            for m in range(t_tok // 128):
                xt = xpool.tile([128, in_dim], f32, tag="x")
                nc.gpsimd.dma_start(xt[:], x[m * 128:(m + 1) * 128, :])
                amax = scal.tile([128, 1], f32, tag="amax")
                nc.vector.tensor_reduce(
                    amax[:], xt[:], axis=mybir.AxisListType.X, op=Alu.max,
                    apply_absolute_value=True)
                asc = scal.tile([128, 1], f32, tag="asc")
                nc.vector.tensor_scalar(
                    asc[:], amax[:], 1e-12, 1.0 / 127.0, Alu.max, Alu.mult)
                inv = scal.tile([128, 1], f32, tag="inv")
                nc.vector.reciprocal(inv[:], asc[:])
                nc.scalar.activation(xt[:], xt[:], Act.Copy, bias=MAGIC,
                                     scale=inv[:])
                xq = xqpool.tile([128, in_dim], bf16, tag="xq")
                nc.vector.tensor_scalar(xq[:], xt[:], MAGIC, None, Alu.subtract)
                xT = xqt.tile([128, KC, 128], bf16, tag="xT")
                for k in range(KC):
                    nc.sync.dma_start(xT[:, k, :], xq[:, k * 128:(k + 1) * 128],
                                      transpose=True)
                ptiles = []
                for _ni in range(len(nsplits)):
                    pst = psum.tile([128, 512], f32, tag="ps")
                    ptiles.append(pst)
                for k in range(KC):
                    for ni, (n0_, nw) in enumerate(nsplits):
                        nc.tensor.matmul(
                            ptiles[ni][:, :nw], xT[:, k, :],
                            wT[:, k, n0_:n0_ + nw],
                            start=(k == 0), stop=(k == KC - 1))
                ot = opool.tile([128, out_c], f32, tag="ot")
                for ni, (n0_, nw) in enumerate(nsplits):
                    nc.scalar.activation(ot[:, n0_:n0_ + nw], ptiles[ni][:, :nw],
                                         Act.Copy, bias=0.0, scale=asc[:])
                nc.gpsimd.dma_start(out[m * 128:(m + 1) * 128, :], ot[:])

        if repeat == 1:
            body()
        else:
            with tc.For_i(0, repeat, 1) as _it:
                body(_it)

    nc.compile()
    return nc


# ---------------------------------------------------------------------------
# SPMD runner: build the jitted 8-core callable once and reuse it.
# ---------------------------------------------------------------------------

_RUNNERS = {}


def make_runner(nc, n_cores=N_CORES):
    import jax
    from jax.sharding import Mesh, PartitionSpec
    from jax.experimental.shard_map import shard_map
    import concourse.mybir as mybir
    from concourse import bass2jax

    bass2jax.install_neuronx_cc_hook()
    in_names, out_names, out_avals = [], [], []
    for alloc in nc.m.functions[0].allocations:
        if not isinstance(alloc, mybir.MemoryLocationSet):
            continue
        name = alloc.memorylocations[0].name
        if alloc.kind == "ExternalInput":
            if nc.partition_id_tensor is None or name != nc.partition_id_tensor.name:
                in_names.append(name)
        elif alloc.kind == "ExternalOutput":
            shape = tuple(alloc.tensor_shape)
            dtype = mybir.dt.np(alloc.dtype)
            out_names.append(name)
            out_avals.append(jax.core.ShapedArray(shape, dtype))
    n_params = len(in_names)
    all_in_names = list(in_names) + list(out_names)
    if nc.partition_id_tensor is not None:
        all_in_names.append(nc.partition_id_tensor.name)

    def _body(*args):
        operands = list(args)
        if nc.partition_id_tensor is not None:
            operands.append(bass2jax.partition_id_tensor())
        outs = bass2jax._bass_exec_p.bind(
            *operands,
            out_avals=tuple(out_avals),
            in_names=tuple(all_in_names),
            out_names=tuple(out_names),
            lowering_input_output_aliases=(),
            sim_require_finite=True,
            sim_require_nnan=True,
            nc=nc,
        )
        return tuple(outs)

    devices = jax.devices()[:n_cores]
    mesh = Mesh(np.asarray(devices), ("core",))
    in_specs = (PartitionSpec("core"),) * (n_params + len(out_names))
    out_specs = (PartitionSpec("core"),) * len(out_names)
    fn = jax.jit(
        shard_map(_body, mesh=mesh, in_specs=in_specs, out_specs=out_specs,
                  check_rep=False),
        keep_unused=True,
    )
    return {
        "fn": fn, "mesh": mesh, "in_names": in_names, "out_names": out_names,
        "out_avals": out_avals, "n_cores": n_cores,
    }


def run_spmd(runner, in_maps):
    """Run the SPMD callable on per-core input dicts; returns per-core output
    dicts."""
    import jax

    n_cores = runner["n_cores"]
    concat_in = [
        np.concatenate([np.asarray(in_maps[c][name]) for c in range(n_cores)],
                       axis=0)
        for name in runner["in_names"]
    ]
    zeros = [
        np.zeros((n_cores * a.shape[0], *a.shape[1:]), a.dtype)
        for a in runner["out_avals"]
    ]
    outs = runner["fn"](*concat_in, *zeros)
    outs = [np.asarray(o) for o in outs]
    per_core = []
    for c in range(n_cores):
        d = {}
        for i, name in enumerate(runner["out_names"]):
            shp = runner["out_avals"][i].shape
            d[name] = outs[i].reshape(n_cores, *shp)[c]
        per_core.append(d)
    return per_core


def shard_inputs(input, weight, scales, zeros):
    x2d = np.ascontiguousarray(
        np.asarray(input, dtype=np.float32).reshape(T_TOK, IN_DIM))
    wpad = np.zeros((OUT_PAD, IN_DIM), np.int8)
    wpad[:OUT_DIM] = np.asarray(weight, dtype=np.int8)
    spad = np.zeros((OUT_PAD, G), np.float32)
    spad[:OUT_DIM] = np.asarray(scales, dtype=np.float32)
    zpad = np.zeros((OUT_PAD, G), np.float32)
    zpad[:OUT_DIM] = np.asarray(zeros, dtype=np.float32)
    in_maps = []
    for c in range(N_CORES):
        lo, hi = c * OUT_C, (c + 1) * OUT_C
        in_maps.append({
            "x": x2d,
            "w": np.ascontiguousarray(wpad[lo:hi]),
            "scales": np.ascontiguousarray(spad[lo:hi]),
            "zeros": np.ascontiguousarray(zpad[lo:hi]),
        })
    return in_maps


def get_runner(repeat=1):
    key = ("full", repeat)
    if key not in _RUNNERS:
        nc = build_bass(repeat=repeat)
        _RUNNERS[key] = make_runner(nc)
    return _RUNNERS[key]


def kernel(input, weight, scales, zeros):
    in_maps = shard_inputs(input, weight, scales, zeros)
    runner = get_runner()
    per_core = run_spmd(runner, in_maps)
    full = np.concatenate([per_core[c]["out"] for c in range(N_CORES)], axis=1)
    out = full[:, :OUT_DIM].reshape(B, S, OUT_DIM)
    return np.ascontiguousarray(out, dtype=np.float32)


# revision 6
# speedup vs baseline: 1.3445x; 1.3445x over previous
"""Int4-weight (groupwise-dequant) linear with dynamic per-token int8 activation
fake-quant, for 8 trn2 NeuronCores.

Math (per reference):
    w_dq[o,i]  = (w[o,i] - zeros[o, i//32]) * scales[o, i//32]
    amax[t]    = max_i |x[t,i]|;  a_scale[t] = max(amax,1e-12)/127
    q[t,i]     = clip(round(x[t,i]/a_scale[t]), -128, 127)
    out[t,o]   = sum_i (q[t,i]*a_scale[t]) * w_dq[o,i]

Sharding: column-parallel over out_features (padded 11008->11264, 1408/core),
activations replicated. Each core computes out[:, c*1408:(c+1)*1408]; host
concatenates and drops the padding.

Device strategy per core:
  - Phase A: dequantize the int8 weight shard to bf16 on DVE (w and zeros/scales
    enter as exact small integers; only the product rounds once to bf16), then
    DMA-xbar-transpose 128x128 blocks into a SBUF-resident [IN, 1408] bf16
    operand laid out K-major for the PE.
  - Phase B (per 128-token tile): load x fp32, DVE abs-max reduce -> a_scale,
    inv = 1/a_scale; ACT computes x*inv + 1.5*2^23 (fp32 magic rounding), DVE
    subtracts the magic and casts to bf16 (exact integers in [-128,127]); DMA
    transposes to K-major; 32 K-chunk matmuls accumulate fp32 PSUM per <=512
    column tile; ACT applies the per-token a_scale while copying PSUM->SBUF;
    DMA out.
The bf16 activation operand is exact, so the only quantization vs the reference
is the single bf16 rounding of w_dq (~2^-10 relative).
"""

import os
import sys

for _p in ("/opt/trn_rl_repo", "/root/.axon_site/_ro/trn_rl_repo"):
    if os.path.isdir(_p) and _p not in sys.path:
        sys.path.append(_p)

import numpy as np

B, S, IN_DIM, OUT_DIM = 4, 2048, 4096, 11008
GROUP = 32
G = IN_DIM // GROUP          # 128 groups per row
N_CORES = 8
OUT_PAD = 11264              # smallest multiple of 128*8 >= 11008
OUT_C = OUT_PAD // N_CORES   # 1408 out features per core
T_TOK = B * S                # 8192 tokens
MAGIC = 12582912.0           # 1.5 * 2**23: fp32 add/sub rounds to nearest int
BATCH_T = True               # one blocked dma-transpose per x tile vs 32 calls


def build_bass(t_tok=T_TOK, out_c=OUT_C, in_dim=IN_DIM, repeat=1, xpool_bufs=3,
               psum_bufs=6):
    import concourse.bacc as bacc
    import concourse.mybir as mybir
    import concourse.tile as tile
    from contextlib import ExitStack

    f32, bf16, i8 = mybir.dt.float32, mybir.dt.bfloat16, mybir.dt.int8
    Alu = mybir.AluOpType
    Act = mybir.ActivationFunctionType

    KC = in_dim // 128            # contraction chunks of 128
    OFB = out_c // 128            # out-feature 128-blocks
    ngrp = in_dim // GROUP
    KQ_SIZE = 1024 if in_dim % 1024 == 0 else in_dim
    KQ = in_dim // KQ_SIZE        # weight-dequant pieces along IN
    nsplits = []
    n0 = 0
    while n0 < out_c:
        nw = min(512, out_c - n0)
        nsplits.append((n0, nw))
        n0 += nw

    nc = bacc.Bacc("TRN2", target_bir_lowering=False, debug=False,
                   num_devices=N_CORES)
    x = nc.dram_tensor("x", [t_tok, in_dim], f32, kind="ExternalInput")
    w = nc.dram_tensor("w", [out_c, in_dim], i8, kind="ExternalInput")
    sc = nc.dram_tensor("scales", [out_c, ngrp], f32, kind="ExternalInput")
    zp = nc.dram_tensor("zeros", [out_c, ngrp], f32, kind="ExternalInput")
    out = nc.dram_tensor("out", [t_tok, out_c], f32, kind="ExternalOutput")

    def bcast_inner(ap, n):
        # append a 0-stride inner dim: [128, g] -> [128, g, n] broadcast
        return ap.to_broadcast((*ap.shape, n))

    with tile.TileContext(nc) as tc, ExitStack() as ctx:
        wres = ctx.enter_context(tc.tile_pool(name="wres", bufs=1))
        wstage = ctx.enter_context(tc.tile_pool(name="wstage", bufs=3))
        qparam = ctx.enter_context(tc.tile_pool(name="qparam", bufs=3))
        wtmp = ctx.enter_context(tc.tile_pool(name="wtmp", bufs=3))
        xpool = ctx.enter_context(tc.tile_pool(name="xpool", bufs=xpool_bufs))
        scal = ctx.enter_context(tc.tile_pool(name="scal", bufs=4))
        xqpool = ctx.enter_context(tc.tile_pool(name="xqp", bufs=2))
        xqt = ctx.enter_context(tc.tile_pool(name="xqt", bufs=2))
        opool = ctx.enter_context(tc.tile_pool(name="opool", bufs=2))
        psum = ctx.enter_context(
            tc.tile_pool(name="psum", bufs=psum_bufs, space="PSUM"))

        def body(_it=None):
            wT = wres.tile([128, KC, out_c], bf16, tag="wT")
            # ---- Phase A: dequantize weight shard, transpose to K-major ----
            for kq in range(KQ):
                gs = KQ_SIZE // GROUP
                g0 = kq * gs
                for of in range(OFB):
                    wi = wstage.tile([128, KQ_SIZE], i8, tag="wi")
                    nc.gpsimd.dma_start(
                        wi[:], w[of * 128:(of + 1) * 128,
                                 kq * KQ_SIZE:(kq + 1) * KQ_SIZE])
                    zt = qparam.tile([128, gs], f32, tag="z")
                    st = qparam.tile([128, gs], f32, tag="s")
                    nc.gpsimd.dma_start(
                        zt[:], zp[of * 128:(of + 1) * 128, g0:g0 + gs])
                    nc.gpsimd.dma_start(
                        st[:], sc[of * 128:(of + 1) * 128, g0:g0 + gs])
                    w1 = wtmp.tile([128, KQ_SIZE], bf16, tag="w1")
                    nc.vector.tensor_tensor(
                        w1[:], wi[:], bcast_inner(zt[:], GROUP), Alu.subtract)
                    w2 = wtmp.tile([128, KQ_SIZE], bf16, tag="w2")
                    nc.vector.tensor_tensor(
                        w2[:], w1[:], bcast_inner(st[:], GROUP), Alu.mult)
                    if BATCH_T:
                        kb = kq * (KQ_SIZE // 128)
                        nc.sync.dma_start(
                            wT[:, kb:kb + KQ_SIZE // 128,
                               of * 128:(of + 1) * 128],
                            w2[:, :], transpose=True)
                    else:
                        for kk in range(KQ_SIZE // 128):
                            k = kq * (KQ_SIZE // 128) + kk
                            nc.sync.dma_start(
                                wT[:, k, of * 128:(of + 1) * 128],
                                w2[:, kk * 128:(kk + 1) * 128], transpose=True)
            # ---- Phase B: token tiles ----
            for m in range(t_tok // 128):
                xt = xpool.tile([128, in_dim], f32, tag="x")
                nc.gpsimd.dma_start(xt[:], x[m * 128:(m + 1) * 128, :])
                amax = scal.tile([128, 1], f32, tag="amax")
                nc.vector.tensor_reduce(
                    amax[:], xt[:], axis=mybir.AxisListType.X, op=Alu.max,
                    apply_absolute_value=True)
                asc = scal.tile([128, 1], f32, tag="asc")
                nc.vector.tensor_scalar(
                    asc[:], amax[:], 1e-12, 1.0 / 127.0, Alu.max, Alu.mult)
                inv = scal.tile([128, 1], f32, tag="inv")
                nc.vector.reciprocal(inv[:], asc[:])
                nc.scalar.activation(xt[:], xt[:], Act.Copy, bias=MAGIC,
                                     scale=inv[:])
                xq = xqpool.tile([128, in_dim], bf16, tag="xq")
                nc.vector.tensor_scalar(xq[:], xt[:], MAGIC, None, Alu.subtract)
                xT = xqt.tile([128, KC, 128], bf16, tag="xT")
                if BATCH_T:
                    nc.sync.dma_start(xT[:, :, :], xq[:, :], transpose=True)
                else:
                    for k in range(KC):
                        nc.sync.dma_start(xT[:, k, :],
                                          xq[:, k * 128:(k + 1) * 128],
                                          transpose=True)
                ptiles = []
                for _ni in range(len(nsplits)):
                    pst = psum.tile([128, 512], f32, tag="ps")
                    ptiles.append(pst)
                for k in range(KC):
                    for ni, (n0_, nw) in enumerate(nsplits):
                        nc.tensor.matmul(
                            ptiles[ni][:, :nw], xT[:, k, :],
                            wT[:, k, n0_:n0_ + nw],
                            start=(k == 0), stop=(k == KC - 1))
                ot = opool.tile([128, out_c], f32, tag="ot")
                for ni, (n0_, nw) in enumerate(nsplits):
                    nc.scalar.activation(ot[:, n0_:n0_ + nw], ptiles[ni][:, :nw],
                                         Act.Copy, bias=0.0, scale=asc[:])
                nc.gpsimd.dma_start(out[m * 128:(m + 1) * 128, :], ot[:])

        if repeat == 1:
            body()
        else:
            with tc.For_i(0, repeat, 1) as _it:
                body(_it)

    nc.compile()
    return nc


# ---------------------------------------------------------------------------
# SPMD runner: build the jitted 8-core callable once and reuse it.
# ---------------------------------------------------------------------------

_RUNNERS = {}


def make_runner(nc, n_cores=N_CORES):
    import jax
    from jax.sharding import Mesh, PartitionSpec
    from jax.experimental.shard_map import shard_map
    import concourse.mybir as mybir
    from concourse import bass2jax

    bass2jax.install_neuronx_cc_hook()
    in_names, out_names, out_avals = [], [], []
    for alloc in nc.m.functions[0].allocations:
        if not isinstance(alloc, mybir.MemoryLocationSet):
            continue
        name = alloc.memorylocations[0].name
        if alloc.kind == "ExternalInput":
            if nc.partition_id_tensor is None or name != nc.partition_id_tensor.name:
                in_names.append(name)
        elif alloc.kind == "ExternalOutput":
            shape = tuple(alloc.tensor_shape)
            dtype = mybir.dt.np(alloc.dtype)
            out_names.append(name)
            out_avals.append(jax.core.ShapedArray(shape, dtype))
    n_params = len(in_names)
    all_in_names = list(in_names) + list(out_names)
    if nc.partition_id_tensor is not None:
        all_in_names.append(nc.partition_id_tensor.name)

    def _body(*args):
        operands = list(args)
        if nc.partition_id_tensor is not None:
            operands.append(bass2jax.partition_id_tensor())
        outs = bass2jax._bass_exec_p.bind(
            *operands,
            out_avals=tuple(out_avals),
            in_names=tuple(all_in_names),
            out_names=tuple(out_names),
            lowering_input_output_aliases=(),
            sim_require_finite=True,
            sim_require_nnan=True,
            nc=nc,
        )
        return tuple(outs)

    devices = jax.devices()[:n_cores]
    mesh = Mesh(np.asarray(devices), ("core",))
    in_specs = (PartitionSpec("core"),) * (n_params + len(out_names))
    out_specs = (PartitionSpec("core"),) * len(out_names)
    fn = jax.jit(
        shard_map(_body, mesh=mesh, in_specs=in_specs, out_specs=out_specs,
                  check_rep=False),
        keep_unused=True,
    )
    return {
        "fn": fn, "mesh": mesh, "in_names": in_names, "out_names": out_names,
        "out_avals": out_avals, "n_cores": n_cores,
    }


def run_spmd(runner, in_maps):
    """Run the SPMD callable on per-core input dicts; returns per-core output
    dicts."""
    import jax

    n_cores = runner["n_cores"]
    concat_in = [
        np.concatenate([np.asarray(in_maps[c][name]) for c in range(n_cores)],
                       axis=0)
        for name in runner["in_names"]
    ]
    zeros = [
        np.zeros((n_cores * a.shape[0], *a.shape[1:]), a.dtype)
        for a in runner["out_avals"]
    ]
    outs = runner["fn"](*concat_in, *zeros)
    outs = [np.asarray(o) for o in outs]
    per_core = []
    for c in range(n_cores):
        d = {}
        for i, name in enumerate(runner["out_names"]):
            shp = runner["out_avals"][i].shape
            d[name] = outs[i].reshape(n_cores, *shp)[c]
        per_core.append(d)
    return per_core


def shard_inputs(input, weight, scales, zeros):
    x2d = np.ascontiguousarray(
        np.asarray(input, dtype=np.float32).reshape(T_TOK, IN_DIM))
    wpad = np.zeros((OUT_PAD, IN_DIM), np.int8)
    wpad[:OUT_DIM] = np.asarray(weight, dtype=np.int8)
    spad = np.zeros((OUT_PAD, G), np.float32)
    spad[:OUT_DIM] = np.asarray(scales, dtype=np.float32)
    zpad = np.zeros((OUT_PAD, G), np.float32)
    zpad[:OUT_DIM] = np.asarray(zeros, dtype=np.float32)
    in_maps = []
    for c in range(N_CORES):
        lo, hi = c * OUT_C, (c + 1) * OUT_C
        in_maps.append({
            "x": x2d,
            "w": np.ascontiguousarray(wpad[lo:hi]),
            "scales": np.ascontiguousarray(spad[lo:hi]),
            "zeros": np.ascontiguousarray(zpad[lo:hi]),
        })
    return in_maps


def get_runner(repeat=1):
    key = ("full", repeat)
    if key not in _RUNNERS:
        nc = build_bass(repeat=repeat)
        _RUNNERS[key] = make_runner(nc)
    return _RUNNERS[key]


def kernel(input, weight, scales, zeros):
    in_maps = shard_inputs(input, weight, scales, zeros)
    runner = get_runner()
    per_core = run_spmd(runner, in_maps)
    full = np.concatenate([per_core[c]["out"] for c in range(N_CORES)], axis=1)
    out = full[:, :OUT_DIM].reshape(B, S, OUT_DIM)
    return np.ascontiguousarray(out, dtype=np.float32)
